# revision 18
# baseline (speedup 1.0000x reference)
"""Gaussian point-cloud rasterization on 8 Trainium2 NeuronCores (Bass/Tile).

Strategy (pixel-sharded, points replicated):
 - 8 cores x 32 image rows each; per core 16 tiles of 512 pixels.
 - Points (N=256) live on partitions in 2 blocks of 128.
 - Depth sort + cumsum-compositing is reformulated as C = S @ a with a
   host-built 0/1 "sorts-before" matrix S (no device sort needed); the
   (1 - acc_before) term uses (I - S) @ a so signs work out with the
   fused DVE ops available.
 - Gaussian log-density is a K=6 matmul of per-point coefficients against
   the per-pixel basis [1, px^2, py^2, px*py, px, py]; opacity and the
   det-normalizer are folded into the constant term, so alpha needs only
   exp + two fused select ops.
 - SH color is a K=16 matmul; sigmoid(x) = 0.5*tanh(x/2)+0.5 so that exp
   and tanh share one ACT table set (no ~2.7us table switches).
 - The 0.5 scale/offset of the tanh trick folds into the PE reduction
   weights (0.5-valued lhsT vectors + one extra accumulating matmul).
"""
import sys
import numpy as np

sys.path.insert(0, "/opt/trn_rl_repo")

N = 256
H = W = 256
NCORES = 8
ROWS = H // NCORES          # 32
PCORE = ROWS * W            # 8192
TILE = 512
NT = PCORE // TILE          # 16
CENTER = 128.0

LN_CLAMP = float(np.float32(np.log(0.99)))        # alpha clamp in logit space
LN_SKIP = float(np.float32(np.log(1.0 / 255.0)))  # alpha skip threshold in logit space
ACC_BREAK = 0.9999

_C0 = 0.28209479177387814
_C1 = 0.4886025119029199
_C2 = (1.0925484305920792, -1.0925484305920792, 0.31539156525252005,
       -1.0925484305920792, 0.5462742152960396)
_C3 = (-0.5900435899266435, 2.890611442640554, -0.4570457994644658, 0.3731763325901154,
       -0.4570457994644658, 1.445305721320277, -0.5900435899266435)

# how many of the 6 per-tile (wgt * tanh) products run on DVE vs GPSIMD
_PROD_ON_VECTOR = (0, 3)


def _host_preprocess(pointcloud, feats, K, T):
    f32 = np.float32
    pc = np.asarray(pointcloud, f32)
    feats = np.asarray(feats, f32)
    K = np.asarray(K, f32)
    T = np.asarray(T, f32)
    R, t = T[:3, :3], T[:3, 3]
    p_cam = pc @ R.T + t
    zc = p_cam[:, 2]
    proj = p_cam @ K.T
    uv = proj[:, :2] / np.clip(zc, 1e-6, None)[:, None]
    in_cam = ((zc > 0.8) & (zc < 1000.0) & (uv[:, 0] >= 0) & (uv[:, 0] < W)
              & (uv[:, 1] >= 0) & (uv[:, 1] < H))
    zs = np.where(in_cam, zc, f32(1e10)).astype(f32)
    idx = np.arange(N)
    # S[i,j] = 1 iff point j composites at-or-before point i under a stable
    # argsort of zs (ties only matter for culled points, which have a = 0)
    S = ((zs[None, :] < zs[:, None])
         | ((zs[None, :] == zs[:, None]) & (idx[None, :] <= idx[:, None]))).astype(f32)
    Sneg = (np.eye(N, dtype=f32) - S).astype(f32)   # (I-S)@a = a - C = -acc_before

    q = feats[:, :4]
    q = q / np.linalg.norm(q, axis=-1, keepdims=True).astype(f32)
    x, y, z, w = q[:, 0], q[:, 1], q[:, 2], q[:, 3]
    Rq = np.stack([
        1 - 2 * (y * y + z * z), 2 * (x * y - z * w), 2 * (x * z + y * w),
        2 * (x * y + z * w), 1 - 2 * (x * x + z * z), 2 * (y * z - x * w),
        2 * (x * z - y * w), 2 * (y * z + x * w), 1 - 2 * (x * x + y * y)],
        axis=-1).reshape(-1, 3, 3).astype(f32)
    s = np.exp(feats[:, 4:7])
    M = Rq * s[:, None, :]
    Sigma = M @ M.transpose(0, 2, 1)
    fx, fy = K[0, 0], K[1, 1]
    zero = np.zeros_like(zc)
    J = np.stack([
        np.stack([fx / zc, zero, -fx * p_cam[:, 0] / (zc * zc)], -1),
        np.stack([zero, fy / zc, -fy * p_cam[:, 1] / (zc * zc)], -1)], axis=-2)
    JW = J @ R
    cov = JW @ Sigma @ JW.transpose(0, 2, 1)
    det = np.maximum(cov[:, 0, 0] * cov[:, 1, 1] - cov[:, 0, 1] * cov[:, 1, 0], 1e-12)
    ia, ib, ic = cov[:, 1, 1] / det, -cov[:, 0, 1] / det, cov[:, 0, 0] / det

    sig_op = 1.0 / (1.0 + np.exp(-feats[:, 7].astype(np.float64)))
    lg = np.log(sig_op) - np.log(2 * np.pi) - 0.5 * np.log(det.astype(np.float64))

    ia64, ib64, ic64 = ia.astype(np.float64), ib.astype(np.float64), ic.astype(np.float64)
    ux = np.clip(uv[:, 0].astype(np.float64) - CENTER, -1e4, 1e4)
    uy = np.clip(uv[:, 1].astype(np.float64) - CENTER, -1e4, 1e4)
    k0 = ia64 * ux * ux + ic64 * uy * uy + 2 * ib64 * ux * uy
    kx = ia64 * ux + ib64 * uy
    ky = ic64 * uy + ib64 * ux
    A = np.stack([lg - 0.5 * k0, -0.5 * ia64, -0.5 * ic64, -ib64, kx, ky]).astype(f32)
    A[0, ~in_cam] = f32(-1e20)

    coeffs = feats[:, 8:56].reshape(N, 3, 16)
    coefft = np.ascontiguousarray(coeffs.transpose(2, 1, 0).reshape(16, 3 * N)).astype(f32)

    wv = np.arange(W, dtype=np.float64) + 0.5 - CENTER
    hv = np.arange(H, dtype=np.float64) + 0.5 - CENTER
    pxg, pyg = np.meshgrid(wv, hv)
    px = pxg.reshape(-1)
    py = pyg.reshape(-1)
    bpix = np.stack([np.ones_like(px), px * px, py * py, px * py, px, py]).astype(f32)

    Kinv = np.linalg.inv(K.astype(np.float64))
    ug, vg = np.meshgrid(np.arange(W, dtype=np.float64), np.arange(H, dtype=np.float64))
    pix = np.stack([ug, vg, np.ones_like(ug)], axis=-1)
    d = (pix @ Kinv.T) @ R.astype(np.float64)
    d = d / np.linalg.norm(d, axis=-1, keepdims=True)
    dx_, dy_, dz_ = d[..., 0], d[..., 1], d[..., 2]
    xx, yy, zz = dx_ * dx_, dy_ * dy_, dz_ * dz_
    shb = np.stack([
        np.full_like(dx_, _C0),
        -_C1 * dy_, _C1 * dz_, -_C1 * dx_,
        _C2[0] * dx_ * dy_, _C2[1] * dy_ * dz_, _C2[2] * (2 * zz - xx - yy),
        _C2[3] * dx_ * dz_, _C2[4] * (xx - yy),
        _C3[0] * dy_ * (3 * xx - yy), _C3[1] * dx_ * dy_ * dz_,
        _C3[2] * dy_ * (4 * zz - xx - yy),
        _C3[3] * dz_ * (2 * zz - 3 * xx - 3 * yy), _C3[4] * dx_ * (4 * zz - xx - yy),
        _C3[5] * dz_ * (xx - yy), _C3[6] * dx_ * (xx - 3 * yy)],
        axis=0).reshape(16, H * W).astype(f32)

    stp = np.zeros((128, 4, 128), f32)
    stn = np.zeros((128, 4, 128), f32)
    for bi in range(2):
        for bj in range(2):
            stp[:, bi * 2 + bj, :] = S[bi * 128:(bi + 1) * 128, bj * 128:(bj + 1) * 128].T
            stn[:, bi * 2 + bj, :] = Sneg[bi * 128:(bi + 1) * 128, bj * 128:(bj + 1) * 128].T

    # reduction weights: slot 4g+0 sums 0.5*wgt into img rows 3g..3g+2,
    # slot 4g+1+c sums 0.5*prod into img row 3g+c (rows of a [12,TILE] psum bank
    # holding 4 consecutive pixel tiles' rgb rows)
    zh = np.zeros((128, 16, 12), f32)
    for g in range(4):
        zh[:, 4 * g + 0, 3 * g:3 * g + 3] = 0.5
        for c in range(3):
            zh[:, 4 * g + 1 + c, 3 * g + c] = 0.5
    return dict(A=A, stp=stp, stn=stn, coefft=coefft, bpix=bpix, shb=shb, zh=zh)


_NC_CACHE = {}


def _build_nc(repeats=1):
    key = ("nc", repeats)
    if key in _NC_CACHE:
        return _NC_CACHE[key]
    from contextlib import ExitStack
    import concourse.tile as tile
    from concourse import bacc, mybir

    f32 = mybir.dt.float32
    op = mybir.AluOpType
    act = mybir.ActivationFunctionType

    nc = bacc.Bacc(None, target_bir_lowering=False, debug=False)
    bpix_d = nc.dram_tensor("bpix", [6, PCORE], f32, kind="ExternalInput")
    shb_d = nc.dram_tensor("shb", [16, PCORE], f32, kind="ExternalInput")
    apr_d = nc.dram_tensor("aprime", [6, N], f32, kind="ExternalInput")
    stp_d = nc.dram_tensor("stpos", [128, 4, 128], f32, kind="ExternalInput")
    stn_d = nc.dram_tensor("stneg", [128, 4, 128], f32, kind="ExternalInput")
    cft_d = nc.dram_tensor("coefft", [16, 3 * N], f32, kind="ExternalInput")
    zh_d = nc.dram_tensor("zh", [128, 16, 12], f32, kind="ExternalInput")
    # [q, 3g+c, n]: channel c of pixel tile ti = 4q+g
    img_d = nc.dram_tensor("img", [NT // 4, 12, TILE], f32, kind="ExternalOutput")

    with tile.TileContext(nc) as tc, ExitStack() as ctx:
        const = ctx.enter_context(tc.tile_pool(name="const", bufs=1))
        work = ctx.enter_context(tc.tile_pool(name="work", bufs=3))
        keep = ctx.enter_context(tc.tile_pool(name="keep", bufs=4))
        ps_q = ctx.enter_context(tc.tile_pool(name="ps_q", bufs=2, space="PSUM"))
        ps_c = ctx.enter_context(tc.tile_pool(name="ps_c", bufs=1, space="PSUM"))
        ps_col = ctx.enter_context(tc.tile_pool(name="ps_col", bufs=2, space="PSUM"))
        ps_img = ctx.enter_context(tc.tile_pool(name="ps_img", bufs=2, space="PSUM"))

        bpix = const.tile([6, PCORE], f32)
        nc.sync.dma_start(out=bpix[:], in_=bpix_d[:])
        shb = const.tile([16, PCORE], f32)
        nc.sync.dma_start(out=shb[:], in_=shb_d[:])
        apr = const.tile([6, N], f32)
        nc.sync.dma_start(out=apr[:], in_=apr_d[:])
        stp = const.tile([128, 4, 128], f32)
        nc.sync.dma_start(out=stp[:], in_=stp_d[:])
        stn = const.tile([128, 4, 128], f32)
        nc.sync.dma_start(out=stn[:], in_=stn_d[:])
        cft = const.tile([16, 3 * N], f32)
        nc.sync.dma_start(out=cft[:], in_=cft_d[:])
        zh = const.tile([128, 16, 12], f32)
        nc.sync.dma_start(out=zh[:], in_=zh_d[:])

        img = None
        for ti_rep in range(NT * repeats):
            ti = ti_rep % NT
            sl = slice(ti * TILE, (ti + 1) * TILE)
            g = ti % 4
            if g == 0:
                img = ps_img.tile([12, TILE], f32, tag="img")
            quads, a_s = [], []
            for b in range(2):
                quad = ps_q.tile([128, TILE], f32, tag="quad")
                nc.tensor.matmul(quad[:], apr[:, b * 128:(b + 1) * 128], bpix[:, sl],
                                 start=True, stop=True)
                t_ = work.tile([128, TILE], f32, tag="t_")
                nc.vector.tensor_scalar(out=t_[:], in0=quad[:], scalar1=LN_CLAMP,
                                        scalar2=None, op0=op.min)
                ex = work.tile([128, TILE], f32, tag="ex")
                nc.scalar.activation(ex[:], t_[:], act.Exp)
                av = keep.tile([128, TILE], f32, tag="av")
                nc.vector.scalar_tensor_tensor(out=av[:], in0=quad[:], scalar=LN_SKIP,
                                               in1=ex[:], op0=op.is_ge, op1=op.mult)
                quads.append(quad)
                a_s.append(av)
            wgts = []
            for b in range(2):
                Cp = ps_c.tile([128, TILE], f32, tag="Cp")
                Cn = ps_c.tile([128, TILE], f32, tag="Cn")
                for bj in range(2):
                    nc.tensor.matmul(Cp[:], stp[:, b * 2 + bj, :], a_s[bj][:],
                                     start=(bj == 0), stop=(bj == 1))
                    nc.tensor.matmul(Cn[:], stn[:, b * 2 + bj, :], a_s[bj][:],
                                     start=(bj == 0), stop=(bj == 1))
                w1 = work.tile([128, TILE], f32, tag="w1")
                nc.vector.scalar_tensor_tensor(out=w1[:], in0=Cn[:], scalar=-1.0,
                                               in1=a_s[b][:], op0=op.subtract, op1=op.mult)
                wgt = keep.tile([128, TILE], f32, tag="wgt")
                nc.vector.scalar_tensor_tensor(out=wgt[:], in0=Cp[:], scalar=ACC_BREAK,
                                               in1=w1[:], op0=op.is_le, op1=op.mult)
                wgts.append(wgt)
            for b in range(2):
                nc.tensor.matmul(img[:], zh[:, 4 * g + 0, :], wgts[b][:],
                                 start=(g == 0 and b == 0), stop=False)
            k = 0
            for c in range(3):
                for b in range(2):
                    col = ps_col.tile([128, TILE], f32, tag="col")
                    nc.tensor.matmul(col[:], cft[:, c * N + b * 128:c * N + (b + 1) * 128],
                                     shb[:, sl], start=True, stop=True)
                    th = work.tile([128, TILE], f32, tag="th")
                    nc.scalar.activation(th[:], col[:], act.Tanh, scale=0.5)
                    prod = work.tile([128, TILE], f32, tag="prod")
                    eng = nc.vector if (k in _PROD_ON_VECTOR) else nc.gpsimd
                    eng.tensor_mul(prod[:], wgts[b][:], th[:])
                    nc.tensor.matmul(img[:], zh[:, 4 * g + 1 + c, :], prod[:],
                                     start=False, stop=(g == 3 and c == 2 and b == 1))
                    k += 1
            if g == 3:
                sbimg = work.tile([12, TILE], f32, tag="sbimg")
                nc.scalar.copy(sbimg[:], img[:])
                nc.sync.dma_start(out=img_d[ti // 4], in_=sbimg[:])
    nc.compile()
    _NC_CACHE[key] = nc
    return nc


def _run(inputs, trace=False, repeats=1):
    from concourse.bass_utils import run_bass_kernel_spmd

    pre = _host_preprocess(inputs["pointcloud"], inputs["pointcloud_features"],
                           inputs["camera_intrinsics"], inputs["T_camera_pointcloud"])
    nc = _build_nc(repeats)
    in_maps = []
    for core in range(NCORES):
        p0 = core * PCORE
        in_maps.append({
            "bpix": np.ascontiguousarray(pre["bpix"][:, p0:p0 + PCORE]),
            "shb": np.ascontiguousarray(pre["shb"][:, p0:p0 + PCORE]),
            "aprime": pre["A"],
            "stpos": pre["stp"],
            "stneg": pre["stn"],
            "coefft": pre["coefft"],
            "zh": pre["zh"],
        })
    bkr = run_bass_kernel_spmd(nc, in_maps, list(range(NCORES)), trace=trace)
    out = np.zeros((H, W, 3), np.float32)
    for core in range(NCORES):
        img = bkr.results[core]["img"]  # [NT//4, 12, TILE]
        flat = np.transpose(img.reshape(NT // 4, 4, 3, TILE), (2, 0, 1, 3)).reshape(3, PCORE)
        out[core * ROWS:(core + 1) * ROWS] = flat.reshape(3, ROWS, W).transpose(1, 2, 0)
    return out, bkr


def kernel(**inputs):
    return _run(inputs)[0]
